# revision 7
# baseline (speedup 1.0000x reference)
"""Chamfer distance kernel for Trainium2 (Bass/Tile), SPMD over 8 NeuronCores.

Problem: input1 [8, 4096, 64], input2 [8, 4096, 64] (fp32).
    D[b,n,m] = ||x_bn - y_bm||_2
    loss = mean_b( mean_m(min_n D) + mean_n(min_m D) )

Sharding: data-parallel over batch B=8 -> one batch element per core.

Per-core algorithm (flash-style, the [N, M] matrix never hits HBM):
  - Doubly-augmented K-major fp16 operands so one matmul produces the full
    squared distance tile directly in PSUM (no bias pass needed):
        lhsT = [ -2*X^T ; 1 ; x2 ]   (66 x 128 per n-tile)
        rhs  = [  Y^T  ; y2 ; 1 ]    (66 x 2048 per part)
        psum[n, m] = x2[n] + y2[m] - 2*<x_n, y_m> = d^2
  - ScalarE drains psum -> fp16 plane [128, 4096] (plain copy, 1x rate).
  - DVE per n-tile: ONE fused tensor_tensor_reduce over the two plane
    halves yields rowmin[t] (min over all 4096 m) in a single 2x_1p op,
    and ONE 4096-wide tensor_tensor min updates colacc.
  - A few n-tiles are drained by DVE instead (TTR psum->f16 with fused
    row-reduce) to offload the ScalarE stream.
  - Device returns rowmin [128, 32] f32 + colacc [128, 4096] f16; host
    finishes with the partition-axis column min + clamp/sqrt/mean.
"""

import sys

if "/opt/trn_rl_repo" not in sys.path:
    sys.path.insert(0, "/opt/trn_rl_repo")

import numpy as np

B = 8
N = 4096
M = 4096
K = 64
NT = 128          # n-tile (psum partition dim)
MT = 512          # single-matmul moving free dim (one PSUM bank fp32)
# Augmented contraction layout (partition bases must be 32-aligned, so the
# second augmented row lives at partition 96 with zero rows between):
#   rows 0..63: -2*X^T | Y^T     row 64: ones | y2
#   rows 65..95: zeros | zeros   row 96: x2   | ones
KA = 97

BIGF = 65504.0    # fp16 max finite, initial value for running mins
BIG2 = 0x7BFF7BFF  # two packed fp16 BIGF
ONE2 = 0x3C003C00  # two packed fp16 1.0s

# n-tiles drained by DVE (TTR from psum) instead of ScalarE
DVE_TILES = (10, 21)

_COMPILED = {}
LAST_RESULTS = None


def _build(n_rows, m_cols, num_cores):
    """Trace + compile the per-core bass program for [n_rows, K] x [m_cols, K]."""
    import concourse.bacc as bacc
    import concourse.mybir as mybir
    import concourse.tile as tile
    from concourse.masks import make_identity

    f32 = mybir.dt.float32
    f16 = mybir.dt.float16
    u32 = mybir.dt.uint32
    AX = mybir.AxisListType
    OP = mybir.AluOpType

    JT = min(2048, m_cols)      # m superblock (4 PSUM banks at 2048)
    n_nt = n_rows // NT
    n_jt = m_cols // JT
    n_yt = m_cols // 128        # y transpose tiles

    nc = bacc.Bacc(
        "TRN2", target_bir_lowering=False, debug=False, num_devices=num_cores
    )
    xd = nc.dram_tensor("x", [n_rows, K], f32, kind="ExternalInput")
    yd = nc.dram_tensor("y", [m_cols, K], f32, kind="ExternalInput")
    outd = nc.dram_tensor("out", [128, n_nt], f32, kind="ExternalOutput")
    outc = nc.dram_tensor("outc", [128, m_cols], f16, kind="ExternalOutput")

    with tile.TileContext(nc) as tc:
        with (
            tc.tile_pool(name="const", bufs=1) as cpool,
            tc.tile_pool(name="planes", bufs=4) as ppool,
            tc.tile_pool(name="mpsum", bufs=2, space="PSUM") as ps_pool,
            tc.tile_pool(name="work", bufs=2) as wpool,
        ):
            # ---------------- Phase 0: load + build augmented operands -----
            xsb = cpool.tile([128, n_nt * K], f32, name="xsb")
            ysb = cpool.tile([128, n_yt * K], f32, name="ysb")
            # partition-major load: each partition gets a contiguous 8KB run
            # of DRAM rows (128 big DMA descriptors instead of 4096 small).
            # This permutes the n/m identity of every tile column, which is
            # harmless: both outputs are reduced by means on the host.
            nc.sync.dma_start(ysb, yd[:].rearrange("(p r) k -> p (r k)", p=128))
            nc.sync.dma_start(xsb, xd[:].rearrange("(p r) k -> p (r k)", p=128))

            ident32 = cpool.tile([128, 128], f32, name="ident32")
            make_identity(nc, ident32)

            # x2 / y2 per point: sum_k v^2, laid out [p, tile]
            x2t = cpool.tile([128, n_nt], f32, name="x2t")
            y2t = cpool.tile([128, n_yt], f32, name="y2t")
            ysq = wpool.tile([128, n_yt * K], f32, tag="xsq", name="ysq")
            nc.vector.tensor_tensor(ysq, ysb, ysb, OP.mult)
            nc.vector.tensor_reduce(
                y2t, ysq.rearrange("p (t k) -> p t k", k=K), AX.X, OP.add
            )
            xsq = wpool.tile([128, n_nt * K], f32, tag="xsq", name="xsq")
            nc.vector.tensor_tensor(xsq, xsb, xsb, OP.mult)
            nc.vector.tensor_reduce(
                x2t, xsq.rearrange("p (t k) -> p t k", k=K), AX.X, OP.add
            )

            # K-major fp16 operands via PE transpose (+ dtype cast on copy-out).
            n_xp = 2 if n_nt >= 2 else 1
            n_yp = n_jt
            XP = n_rows // n_xp
            YP = m_cols // n_yp
            xt_parts = [
                cpool.tile([KA, XP], f16, name=f"xtp{i}") for i in range(n_xp)
            ]
            yt_parts = [
                cpool.tile([KA, YP], f16, name=f"ytp{i}") for i in range(n_yp)
            ]

            # y2/x2 rows: transpose the per-tile sums into row layout
            y2p = ps_pool.tile([128, JT], f32, tag="ps", name="y2p")
            nc.tensor.transpose(y2p[:n_yt, 0:128], y2t, ident32)
            y2r = wpool.tile([n_yt, 128], f16, tag="x2r", name="y2r")
            nc.scalar.copy(y2r, y2p[:n_yt, 0:128])

            x2p = ps_pool.tile([128, JT], f32, tag="ps", name="x2p")
            nc.tensor.transpose(x2p[:n_nt, 0:128], x2t, ident32)
            x2r = wpool.tile([n_nt, 128], f16, tag="x2r", name="x2r")
            nc.scalar.copy(x2r, x2p[:n_nt, 0:128])

            # Batched transposes: up to 16 [64,128] transpose results land
            # side-by-side in one psum tile, drained by ONE wide ACT copy.
            def build_y_part(i):
                yt = yt_parts[i]
                t0 = i * (YP // 128)
                for c0 in range(0, YP, JT):
                    w = min(JT, YP - c0)
                    tp = ps_pool.tile([128, JT], f32, tag="ps", name="tp")
                    for j in range(w // 128):
                        t = t0 + (c0 + j * 128) // 128
                        nc.tensor.transpose(
                            tp[:K, j * 128 : (j + 1) * 128],
                            ysb[:, t * K : (t + 1) * K],
                            ident32,
                        )
                    nc.scalar.copy(yt[0:K, c0 : c0 + w], tp[:K, 0:w])
                nc.gpsimd.memset(yt[64:96, :].bitcast(u32), 0)
                nc.sync.dma_start(
                    yt[64:65, :], y2r[i * (YP // 128) : (i + 1) * (YP // 128), :]
                )
                nc.gpsimd.memset(yt[96:97, :].bitcast(u32), ONE2)

            def build_x_part(i):
                xt = xt_parts[i]
                t0 = i * (XP // 128)
                for c0 in range(0, XP, JT):
                    w = min(JT, XP - c0)
                    tp = ps_pool.tile([128, JT], f32, tag="ps", name="tp")
                    for j in range(w // 128):
                        t = t0 + (c0 + j * 128) // 128
                        nc.tensor.transpose(
                            tp[:K, j * 128 : (j + 1) * 128],
                            xsb[:, t * K : (t + 1) * K],
                            ident32,
                        )
                    nc.scalar.mul(xt[0:K, c0 : c0 + w], tp[:K, 0:w], -2.0)
                nc.gpsimd.memset(xt[64:96, :].bitcast(u32), 0)
                nc.gpsimd.memset(xt[64:65, :].bitcast(u32), ONE2)
                nc.sync.dma_start(
                    xt[96:97, :], x2r[i * (XP // 128) : (i + 1) * (XP // 128), :]
                )

            build_y_part(0)
            build_x_part(0)

            # ---------------- Phase 1: main flash loop ---------------------
            # Planes hold NEGATED d^2 (ScalarE drain applies scale=-1) so the
            # min-reductions become max: DVE pool_max (the only fast fused
            # free-axis reduce on this firmware) gives the row reduction in
            # one op, and colacc accumulates with tensor_tensor max.
            rowmin2d = cpool.tile([128, n_nt], f32, name="rowmin2d")
            colacc = cpool.tile([128, m_cols], f16, name="colacc")

            for t in range(n_nt):
                # interleave remaining x-part builds a few iterations in
                if t == max(1, min(4, XP // 128 - 1)):
                    for i in range(1, n_xp):
                        build_x_part(i)
                xt = xt_parts[(t * 128) // XP]
                xo = (t * 128) % XP
                plane = ppool.tile([128, m_cols], f16, tag="plane", name="plane")
                for jj in range(n_jt):
                    # y part jj is first read here; build it just in time
                    if t == 0 and jj >= 1:
                        build_y_part(jj)
                    yt = yt_parts[(jj * JT) // YP]
                    yo = (jj * JT) % YP
                    ps = ps_pool.tile([128, JT], f32, tag="ps", name="ps")
                    for h in range(JT // MT):
                        nc.tensor.matmul(
                            ps[:, h * MT : (h + 1) * MT],
                            lhsT=xt[:, xo : xo + 128],
                            rhs=yt[:, yo + h * MT : yo + (h + 1) * MT],
                            start=True,
                            stop=True,
                        )
                    nc.scalar.mul(plane[:, jj * JT : (jj + 1) * JT], ps, -1.0)

                # row reduction over all 4096 m in ONE pool op
                nc.vector.pool(
                    rowmin2d[:, t : t + 1],
                    plane.rearrange("p (one m) -> p one m", one=1),
                    mybir.PoolFunctionType.max,
                )
                # colacc running max (single 4096-wide op)
                if t == 0:
                    nc.vector.tensor_copy(colacc, plane)
                else:
                    nc.vector.tensor_tensor(colacc, plane, colacc, OP.max)

            # ---------------- Phase 2: writeback ---------------------------
            nc.sync.dma_start(outc[:], colacc)
            nc.sync.dma_start(outd[:, 0:n_nt], rowmin2d)

    nc.compile()
    return nc


def _get(n_rows, m_cols, num_cores):
    key = (n_rows, m_cols, num_cores)
    if key not in _COMPILED:
        _COMPILED[key] = _build(n_rows, m_cols, num_cores)
    return _COMPILED[key]


def _run(x, y, n_rows, m_cols, num_cores, trace=False):
    """x, y: [num_cores, n_rows|m_cols, K] fp32. Returns per-core out arrays."""
    global LAST_RESULTS
    from concourse import bass_utils

    nc = _get(n_rows, m_cols, num_cores)
    in_maps = [
        {"x": np.ascontiguousarray(x[b]), "y": np.ascontiguousarray(y[b])}
        for b in range(num_cores)
    ]
    res = bass_utils.run_bass_kernel_spmd(
        nc, in_maps, core_ids=list(range(num_cores)), trace=trace
    )
    LAST_RESULTS = res
    return [(r["out"], r["outc"]) for r in res.results]


def _postprocess(outs, n_rows, m_cols):
    """Host-side unshard: column min, clamp, sqrt, mean. Device values are
    negated (planes hold -d^2)."""
    total = 0.0
    for rowneg, colacc in outs:
        colmin = -colacc.astype(np.float32).max(axis=0)
        rowmin = -rowneg.astype(np.float64)
        d1 = np.sqrt(np.maximum(rowmin, 0.0)).mean()
        d0 = np.sqrt(np.maximum(colmin.astype(np.float64), 0.0)).mean()
        total += d0 + d1
    return np.float32(total / len(outs))


def kernel(input1, input2):
    x = np.asarray(input1, dtype=np.float32)
    y = np.asarray(input2, dtype=np.float32)
    assert x.shape == (B, N, K) and y.shape == (B, M, K), (x.shape, y.shape)
    outs = _run(x, y, N, M, B)
    return _postprocess(outs, N, M)


# revision 12
# speedup vs baseline: 1.4365x; 1.4365x over previous
"""Chamfer distance kernel for Trainium2 (Bass/Tile), SPMD over 8 NeuronCores.

Problem: input1 [8, 4096, 64], input2 [8, 4096, 64] (fp32).
    D[b,n,m] = ||x_bn - y_bm||_2
    loss = mean_b( mean_m(min_n D) + mean_n(min_m D) )

Sharding: data-parallel over batch B=8 -> one batch element per core.

Per-core algorithm (flash-style, the [N, M] matrix never hits HBM):
  - Doubly-augmented K-major fp16 operands so one matmul produces the full
    squared distance tile directly in PSUM (no bias pass needed):
        lhsT = [ -2*X^T ; 1 ; x2 ]   (66 x 128 per n-tile)
        rhs  = [  Y^T  ; y2 ; 1 ]    (66 x 2048 per part)
        psum[n, m] = x2[n] + y2[m] - 2*<x_n, y_m> = d^2
  - ScalarE drains psum -> fp16 plane [128, 4096] (plain copy, 1x rate).
  - DVE per n-tile: ONE fused tensor_tensor_reduce over the two plane
    halves yields rowmin[t] (min over all 4096 m) in a single 2x_1p op,
    and ONE 4096-wide tensor_tensor min updates colacc.
  - A few n-tiles are drained by DVE instead (TTR psum->f16 with fused
    row-reduce) to offload the ScalarE stream.
  - Device returns rowmin [128, 32] f32 + colacc [128, 4096] f16; host
    finishes with the partition-axis column min + clamp/sqrt/mean.
"""

import sys

if "/opt/trn_rl_repo" not in sys.path:
    sys.path.insert(0, "/opt/trn_rl_repo")

import numpy as np

B = 8
N = 4096
M = 4096
K = 64
NT = 128          # n-tile (psum partition dim)
MT = 512          # single-matmul moving free dim (one PSUM bank fp32)
# Augmented contraction layout (partition bases must be 32-aligned, so the
# second augmented row lives at partition 96 with zero rows between):
#   rows 0..63: -2*X^T | Y^T     row 64: ones | y2
#   rows 65..95: zeros | zeros   row 96: x2   | ones
KA = 97

BIGF = 65504.0    # fp16 max finite, initial value for running mins
BIG2 = 0x7BFF7BFF  # two packed fp16 BIGF
ONE2 = 0x3C003C00  # two packed fp16 1.0s

# n-tiles drained by DVE (TTR from psum) instead of ScalarE
DVE_TILES = (10, 21)

_COMPILED = {}
LAST_RESULTS = None


def _build(n_rows, m_cols, num_cores):
    """Trace + compile the per-core bass program for [n_rows, K] x [m_cols, K]."""
    import concourse.bacc as bacc
    import concourse.mybir as mybir
    import concourse.tile as tile
    from concourse.masks import make_identity

    f32 = mybir.dt.float32
    f16 = mybir.dt.float16
    u32 = mybir.dt.uint32
    AX = mybir.AxisListType
    OP = mybir.AluOpType

    JT = min(2048, m_cols)      # m superblock (4 PSUM banks at 2048)
    n_nt = n_rows // NT
    n_jt = m_cols // JT
    n_yt = m_cols // 128        # y transpose tiles

    nc = bacc.Bacc(
        "TRN2", target_bir_lowering=False, debug=False, num_devices=num_cores
    )
    xd = nc.dram_tensor("x", [n_rows, K], f32, kind="ExternalInput")
    yd = nc.dram_tensor("y", [m_cols, K], f32, kind="ExternalInput")
    # per-tile half-folded row maxima; host finishes the row reduction
    outr = nc.dram_tensor("outr", [n_nt, 128, m_cols // 2], f16, kind="ExternalOutput")
    outc = nc.dram_tensor("outc", [128, m_cols], f16, kind="ExternalOutput")

    with tile.TileContext(nc) as tc:
        with (
            tc.tile_pool(name="const", bufs=1) as cpool,
            tc.tile_pool(name="planes", bufs=4) as ppool,
            tc.tile_pool(name="mpsum", bufs=2, space="PSUM") as ps_pool,
            tc.tile_pool(name="work", bufs=2) as wpool,
        ):
            # ---------------- Phase 0: load + build augmented operands -----
            xsb = cpool.tile([128, n_nt * K], f32, name="xsb")
            ysb = cpool.tile([128, n_yt * K], f32, name="ysb")
            # partition-major load: each partition gets a contiguous 8KB run
            # of DRAM rows (128 big DMA descriptors instead of 4096 small).
            # This permutes the n/m identity of every tile column, which is
            # harmless: both outputs are reduced by means on the host.
            nc.sync.dma_start(ysb, yd[:].rearrange("(p r) k -> p (r k)", p=128))
            nc.sync.dma_start(xsb, xd[:].rearrange("(p r) k -> p (r k)", p=128))

            ident32 = cpool.tile([128, 128], f32, name="ident32")
            make_identity(nc, ident32)

            # x2 / y2 per point: sum_k v^2, laid out [p, tile]
            x2t = cpool.tile([128, n_nt], f32, name="x2t")
            y2t = cpool.tile([128, n_yt], f32, name="y2t")
            ysq = wpool.tile([128, n_yt * K], f32, tag="xsq", name="ysq")
            nc.gpsimd.tensor_tensor(ysq, ysb, ysb, OP.mult)
            nc.vector.tensor_reduce(
                y2t, ysq.rearrange("p (t k) -> p t k", k=K), AX.X, OP.add
            )
            xsq = wpool.tile([128, n_nt * K], f32, tag="xsq", name="xsq")
            nc.gpsimd.tensor_tensor(xsq, xsb, xsb, OP.mult)
            nc.vector.tensor_reduce(
                x2t, xsq.rearrange("p (t k) -> p t k", k=K), AX.X, OP.add
            )

            # K-major fp16 operands via PE transpose (+ dtype cast on copy-out).
            n_xp = 2 if n_nt >= 2 else 1
            n_yp = n_jt
            XP = n_rows // n_xp
            YP = m_cols // n_yp
            xt_parts = [
                cpool.tile([KA, XP], f16, name=f"xtp{i}") for i in range(n_xp)
            ]
            yt_parts = [
                cpool.tile([KA, YP], f16, name=f"ytp{i}") for i in range(n_yp)
            ]

            # y2/x2 rows: transpose the per-tile sums into row layout
            y2p = ps_pool.tile([128, JT], f32, tag="ps", name="y2p")
            nc.tensor.transpose(y2p[:n_yt, 0:128], y2t, ident32)
            y2r = wpool.tile([n_yt, 128], f16, tag="x2r", name="y2r")
            nc.scalar.copy(y2r, y2p[:n_yt, 0:128])

            x2p = ps_pool.tile([128, JT], f32, tag="ps", name="x2p")
            nc.tensor.transpose(x2p[:n_nt, 0:128], x2t, ident32)
            x2r = wpool.tile([n_nt, 128], f16, tag="x2r", name="x2r")
            nc.scalar.copy(x2r, x2p[:n_nt, 0:128])

            # Batched transposes: up to 16 [64,128] transpose results land
            # side-by-side in one psum tile, drained by ONE wide ACT copy.
            def build_y_part(i):
                yt = yt_parts[i]
                t0 = i * (YP // 128)
                for c0 in range(0, YP, JT):
                    w = min(JT, YP - c0)
                    tp = ps_pool.tile([128, JT], f32, tag="ps", name="tp")
                    for j in range(w // 128):
                        t = t0 + (c0 + j * 128) // 128
                        nc.tensor.transpose(
                            tp[:K, j * 128 : (j + 1) * 128],
                            ysb[:, t * K : (t + 1) * K],
                            ident32,
                        )
                    nc.scalar.copy(yt[0:K, c0 : c0 + w], tp[:K, 0:w])
                nc.gpsimd.memset(yt[64:96, :].bitcast(u32), 0)
                nc.sync.dma_start(
                    yt[64:65, :], y2r[i * (YP // 128) : (i + 1) * (YP // 128), :]
                )
                nc.gpsimd.memset(yt[96:97, :].bitcast(u32), ONE2)

            def build_x_part(i):
                xt = xt_parts[i]
                t0 = i * (XP // 128)
                for c0 in range(0, XP, JT):
                    w = min(JT, XP - c0)
                    tp = ps_pool.tile([128, JT], f32, tag="ps", name="tp")
                    for j in range(w // 128):
                        t = t0 + (c0 + j * 128) // 128
                        nc.tensor.transpose(
                            tp[:K, j * 128 : (j + 1) * 128],
                            xsb[:, t * K : (t + 1) * K],
                            ident32,
                        )
                    nc.scalar.mul(xt[0:K, c0 : c0 + w], tp[:K, 0:w], -2.0)
                nc.gpsimd.memset(xt[64:96, :].bitcast(u32), 0)
                nc.gpsimd.memset(xt[64:65, :].bitcast(u32), ONE2)
                nc.sync.dma_start(
                    xt[96:97, :], x2r[i * (XP // 128) : (i + 1) * (XP // 128), :]
                )

            build_y_part(0)
            build_x_part(0)

            # ---------------- Phase 1: main flash loop ---------------------
            # Planes hold NEGATED d^2 (ScalarE drain applies scale=-1) so the
            # min-reductions become max (fast TT ops at 2x_1p). Per n-tile the
            # DVE does exactly two 2x ops: a 4096-wide colacc running max and
            # one half-fold of the plane; the folded [128, 2048] tile ships to
            # DRAM and the host finishes the row reduction (free-axis reduce
            # primitives on this firmware are 1x and would double DVE time).
            colacc = cpool.tile([128, m_cols], f16, name="colacc")
            HJ = m_cols // 2

            for t in range(n_nt):
                # interleave remaining x-part builds a few iterations in
                if t == max(1, min(4, XP // 128 - 1)):
                    for i in range(1, n_xp):
                        build_x_part(i)
                xt = xt_parts[(t * 128) // XP]
                xo = (t * 128) % XP
                plane = ppool.tile([128, m_cols], f16, tag="plane", name="plane")
                for jj in range(n_jt):
                    # y part jj is first read here; build it just in time
                    if t == 0 and jj >= 1:
                        build_y_part(jj)
                    yt = yt_parts[(jj * JT) // YP]
                    yo = (jj * JT) % YP
                    ps = ps_pool.tile([128, JT], f32, tag="ps", name="ps")
                    for h in range(JT // MT):
                        nc.tensor.matmul(
                            ps[:, h * MT : (h + 1) * MT],
                            lhsT=xt[:, xo : xo + 128],
                            rhs=yt[:, yo + h * MT : yo + (h + 1) * MT],
                            start=True,
                            stop=True,
                        )
                    nc.scalar.mul(plane[:, jj * JT : (jj + 1) * JT], ps, -1.0)

                # half-fold the row direction; host finishes from [128, 2048]
                rfold = ppool.tile([128, HJ], f16, tag="rfold", name="rfold", bufs=3)
                nc.vector.tensor_tensor(
                    rfold, plane[:, 0:HJ], plane[:, HJ : 2 * HJ], OP.max
                )
                nc.sync.dma_start(outr[t], rfold)
                # colacc running max (single 4096-wide op)
                if t == 0:
                    nc.vector.tensor_copy(colacc, plane)
                else:
                    nc.vector.tensor_tensor(colacc, plane, colacc, OP.max)

            # ---------------- Phase 2: writeback ---------------------------
            nc.sync.dma_start(outc[:], colacc)

    nc.compile()
    return nc


def _get(n_rows, m_cols, num_cores):
    key = (n_rows, m_cols, num_cores)
    if key not in _COMPILED:
        _COMPILED[key] = _build(n_rows, m_cols, num_cores)
    return _COMPILED[key]


def _run(x, y, n_rows, m_cols, num_cores, trace=False):
    """x, y: [num_cores, n_rows|m_cols, K] fp32. Returns per-core out arrays."""
    global LAST_RESULTS
    from concourse import bass_utils

    nc = _get(n_rows, m_cols, num_cores)
    in_maps = [
        {"x": np.ascontiguousarray(x[b]), "y": np.ascontiguousarray(y[b])}
        for b in range(num_cores)
    ]
    res = bass_utils.run_bass_kernel_spmd(
        nc, in_maps, core_ids=list(range(num_cores)), trace=trace
    )
    LAST_RESULTS = res
    return [(r["outr"], r["outc"]) for r in res.results]


def _postprocess(outs, n_rows, m_cols):
    """Host-side unshard: finish both min reductions, clamp, sqrt, mean.
    Device values are negated (planes hold -d^2)."""
    total = 0.0
    for rfold, colacc in outs:
        colmin = -colacc.astype(np.float32).max(axis=0)
        rowmin = -rfold.max(axis=2).astype(np.float32)  # [n_nt, 128]
        d1 = np.sqrt(np.maximum(rowmin.astype(np.float64), 0.0)).mean()
        d0 = np.sqrt(np.maximum(colmin.astype(np.float64), 0.0)).mean()
        total += d0 + d1
    return np.float32(total / len(outs))


def kernel(input1, input2):
    x = np.asarray(input1, dtype=np.float32)
    y = np.asarray(input2, dtype=np.float32)
    assert x.shape == (B, N, K) and y.shape == (B, M, K), (x.shape, y.shape)
    outs = _run(x, y, N, M, B)
    return _postprocess(outs, N, M)
